# revision 1
# baseline (speedup 1.0000x reference)
"""Trainium2 Bass kernel for CustomBertSelfAttention (no head split).

reference:
    q = hs @ Wq + bq; k = hs @ Wk + bk; v = hs @ Wv + bv        # [B,S,D]
    scores = (q @ k^T) / sqrt(64) + mask                         # [B,S,S]
    probs  = softmax(scores, -1)
    out    = probs @ v                                           # [B,S,D]

B=8, S=2048, D=1024.  Sharding: data-parallel over batch, one batch
element per NeuronCore (8 cores), no collectives.

Per-core plan (all matmuls in fp32r = TF32-like dtype, full PE rate):
  1a. hs -> hsT [d, s] via PE transpose mode (after ~3.5us of junk
      matmuls to warm the HAM clock gate), PSUM -> SBUF copies cast to
      fp32r on DVE.
  1b. projections with contraction d on partitions:
        kT[dout, s] (SBUF resident), qT[dout, s] -> DRAM spill,
        v[t, d] natural -> DRAM spill (spills on the gpsimd queue so the
        sync queue stays clear for phase-2 loads).
  2.  per s-block of 256 columns:
        scoresT[t, s] = sum_dk matmuls, kT chunks stationary (PSUM fp32)
        exp on ACT: exp(scores*0.125 + mask[t]) -> SBUF fp32r
        rowsum over t: ones-vector-stationary matmuls -> rowsumT [1, s],
        then per-128 PE transposes ([1,1] identity) + DVE reciprocal
        context[s, d] = sum_tc expT-chunk @ v-chunk (PSUM)
        normalize via tensor_scalar_mul on the PSUM->SBUF copy, DMA out.

Known DMA pitfalls baked in: 4-byte-scatter / broadcast constant loads
(mask/biases) are slow DIRECT2D patterns and sit behind all hs chunks in
the sync queue; the fp32r producer rule and the even-moving-dim rule for
fp32r matmuls are documented in the project memory.
"""

import sys

sys.path.insert(0, "/opt/trn_rl_repo")

from contextlib import ExitStack

import numpy as np

import concourse.bass as bass
import concourse.mybir as mybir
import concourse.tile as tile
from concourse import bacc
from concourse.bass_utils import run_bass_kernel_spmd
from concourse.masks import make_identity

B, S, D = 8, 2048, 1024
NCORES = 8
PD = 128            # partition dim
DK = D // PD        # 8 contraction chunks
SC = S // PD        # 16 sequence chunks
NT = 512            # matmul moving-dim tile (one PSUM bank of fp32)
SBLK = 256          # attention s-block
NBLK = S // SBLK
F32 = mybir.dt.float32
F32R = mybir.dt.float32r
EXP = mybir.ActivationFunctionType.Exp

_compiled_nc = None


def _build():
    nc = bacc.Bacc(
        "TRN2",
        target_bir_lowering=False,
        debug=False,
        num_devices=NCORES,
        enable_asserts=False,
    )
    hs = nc.dram_tensor("hidden_states", [S, D], F32, kind="ExternalInput").ap()
    mask = nc.dram_tensor("attention_mask", [1, S], F32, kind="ExternalInput").ap()
    Wq = nc.dram_tensor("Wq", [D, D], F32, kind="ExternalInput").ap()
    Wk = nc.dram_tensor("Wk", [D, D], F32, kind="ExternalInput").ap()
    Wv = nc.dram_tensor("Wv", [D, D], F32, kind="ExternalInput").ap()
    bq = nc.dram_tensor("bq", [D], F32, kind="ExternalInput").ap()
    bk = nc.dram_tensor("bk", [D], F32, kind="ExternalInput").ap()
    bv = nc.dram_tensor("bv", [D], F32, kind="ExternalInput").ap()
    out = nc.dram_tensor("context", [S, D], F32, kind="ExternalOutput").ap()

    with tile.TileContext(nc) as tc, ExitStack() as ctx:
        persist = ctx.enter_context(tc.tile_pool(name="persist", bufs=1))
        dramp = ctx.enter_context(tc.tile_pool(name="dram", bufs=1, space="DRAM"))
        qT_dram = dramp.tile([D, S], F32R)
        v_dram = dramp.tile([S, D], F32R)

        kT = persist.tile([PD, DK, S], F32R)

        # mask[t] laid out [p, tc] so bias slice [:, tc] is per-partition.
        # DMAs for these constants are emitted later (the hs loads must be
        # first in the sync queue; the bv broadcast alone is an 11.5us
        # DIRECT2D replication that would stall kernel start).
        mask_sb = persist.tile([PD, SC], F32)
        bq_sb = persist.tile([PD, DK], F32)
        bk_sb = persist.tile([PD, DK], F32)
        bv_row = persist.tile([PD, D], F32)

        ident = persist.tile([PD, PD], F32)
        make_identity(nc, ident)
        # fp32r matmuls need an even moving-dim count (2 results/cycle),
        # so the rowsum uses a [PD, 2] ones operand and a [PD, 2] psum.
        ones32 = persist.tile([PD, 2], F32)
        nc.vector.memset(ones32, 1.0)
        ones_r = persist.tile([PD, 2], F32R)
        nc.vector.tensor_copy(out=ones_r, in_=ones32)

        with ExitStack() as p1:
            hstp = p1.enter_context(tc.tile_pool(name="hsT_pool", bufs=1))
            # 4 column-tiles (one per 512-wide s-tile) so projections can
            # start as soon as their columns are transposed.
            hsT_st = [
                hstp.tile([PD, DK, NT], F32R, name=f"hsT{st}", tag=f"hsT{st}")
                for st in range(S // NT)
            ]

            def hsT(dk, lo, hi):
                st, off = lo // NT, lo % NT
                assert hi - lo <= NT and hi <= (st + 1) * NT
                return hsT_st[st][:, dk, off : off + (hi - lo)]

            # Wv is the one full-size weight load; issued after the first
            # couple of hs chunks (so the gpsimd Q7 clears the kernel
            # preamble sync first), still ~100us before the v projection.
            wvp = p1.enter_context(tc.tile_pool(name="wvp", bufs=1))
            wv = wvp.tile([PD, DK, D], F32R)

            # ---- phase 1a: hs -> hsT (PE fast-transpose mode)
            with (
                tc.tile_pool(name="hsload", bufs=6) as hsp,
                tc.tile_pool(name="ptr", bufs=6, space="PSUM") as ptr,
            ):
                # ~3.5us of junk fp32 matmuls while the first hs chunk loads:
                # transpose-mode doesn't count as PE-busy for the HAM clock
                # gate, so without this the whole transpose phase runs at the
                # cold 1.2 GHz rate.  DMA-out so DCE keeps it.
                warm_ps = ptr.tile([PD, PD], F32, name="warm_ps", tag="warm_ps", bufs=1)
                for _ in range(8):
                    nc.tensor.matmul(
                        out=warm_ps, lhsT=ident, rhs=ident, start=True, stop=True
                    )
                warm_sb = hsp.tile([PD, PD], F32, name="warm_sb", tag="warm_sb", bufs=1)
                nc.vector.tensor_copy(out=warm_sb, in_=warm_ps)
                warm_dram = dramp.tile([PD, PD], F32, name="warm_dram", tag="warm_dram")
                nc.sync.dma_start(out=warm_dram[:, :], in_=warm_sb)
                for sc in range(SC):
                    hchunk = hsp.tile([PD, D], F32)
                    nc.sync.dma_start(out=hchunk, in_=hs[sc * PD : (sc + 1) * PD, :])
                    if sc == 2:
                        nc.gpsimd.dma_start(
                            out=wv, in_=Wv.rearrange("(dk p) n -> p dk n", p=PD)
                        )
                    if sc == SC - 1:
                        # 4-byte-scatter / broadcast constant loads are slow
                        # DIRECT2D patterns (mask alone ~14us) - keep them
                        # behind all 16 hs chunks in the sync queue.
                        nc.sync.dma_start(
                            out=bk_sb, in_=bk.rearrange("(c p) -> p c", p=PD)
                        )
                        nc.sync.dma_start(
                            out=bq_sb, in_=bq.rearrange("(c p) -> p c", p=PD)
                        )
                        nc.sync.dma_start(
                            out=mask_sb,
                            in_=mask[0, :].rearrange("(c p) -> p c", p=PD),
                        )
                        bv_bcast = bass.AP(
                            tensor=bv.tensor, offset=bv.offset, ap=[[0, PD], *bv.ap]
                        )
                        nc.sync.dma_start(out=bv_row, in_=bv_bcast)
                    for dk in range(DK):
                        pst = ptr.tile([PD, PD], F32)
                        nc.tensor.transpose(
                            out=pst,
                            in_=hchunk[:, dk * PD : (dk + 1) * PD],
                            identity=ident,
                        )
                        nc.vector.tensor_copy(
                            out=hsT(dk, sc * PD, (sc + 1) * PD), in_=pst
                        )

            # ---- phase 1b: projections
            def project_qk(W, bias_sb, writer):
                with (
                    tc.tile_pool(name="wp", bufs=3) as wp,
                    tc.tile_pool(name="pp", bufs=3, space="PSUM") as pp,
                ):
                    for m in range(DK):
                        wm = wp.tile([PD, DK, PD], F32R)
                        nc.gpsimd.dma_start(
                            out=wm,
                            in_=W[:, m * PD : (m + 1) * PD].rearrange(
                                "(dk p) j -> p dk j", p=PD
                            ),
                        )
                        for st in range(S // NT):
                            ps = pp.tile([PD, NT], F32)
                            for dk in range(DK):
                                nc.tensor.matmul(
                                    out=ps,
                                    lhsT=wm[:, dk, :],
                                    rhs=hsT(dk, st * NT, (st + 1) * NT),
                                    start=(dk == 0),
                                    stop=(dk == DK - 1),
                                )
                            writer(m, st, ps)

            # k projection: straight into resident kT
            def k_writer(m, st, ps):
                nc.vector.tensor_scalar_add(
                    out=kT[:, m, st * NT : (st + 1) * NT],
                    in0=ps,
                    scalar1=bk_sb[:, m : m + 1],
                )

            project_qk(Wk, bk_sb, k_writer)

            # q projection: stage per m-chunk, spill to DRAM
            with tc.tile_pool(name="qstage", bufs=2) as qsp:
                qstages = {}

                def q_writer(m, st, ps):
                    if st == 0:
                        qstages[m] = qsp.tile(
                            [PD, S], F32R, name="qstage_t", tag="qstage_t"
                        )
                    nc.vector.tensor_scalar_add(
                        out=qstages[m][:, st * NT : (st + 1) * NT],
                        in0=ps,
                        scalar1=bq_sb[:, m : m + 1],
                    )
                    if st == S // NT - 1:
                        nc.sync.dma_start(
                            out=qT_dram[m * PD : (m + 1) * PD, :], in_=qstages[m]
                        )

                project_qk(Wq, bq_sb, q_writer)

            # v projection: natural [t, d] layout, spill to DRAM
            # (spills go via gpsimd/SWDGE so the sync queue stays clear for
            # the first attention q-slice + v reload)
            with (
                tc.tile_pool(name="pv", bufs=3, space="PSUM") as pv,
                tc.tile_pool(name="vstage", bufs=2) as vsp,
            ):
                for tcn in range(SC):
                    vstage = vsp.tile([PD, D], F32R)
                    for dt in range(D // NT):
                        ps = pv.tile([PD, NT], F32)
                        for dk in range(DK):
                            nc.tensor.matmul(
                                out=ps,
                                lhsT=hsT(dk, tcn * PD, (tcn + 1) * PD),
                                rhs=wv[:, dk, dt * NT : (dt + 1) * NT],
                                start=(dk == 0),
                                stop=(dk == DK - 1),
                            )
                        nc.vector.tensor_add(
                            out=vstage[:, dt * NT : (dt + 1) * NT],
                            in0=ps,
                            in1=bv_row[:, dt * NT : (dt + 1) * NT],
                        )
                    nc.gpsimd.dma_start(
                        out=v_dram[tcn * PD : (tcn + 1) * PD, :], in_=vstage
                    )

        # ---- phase 2: attention
        with (
            tc.tile_pool(name="vsb", bufs=1) as vp,
            tc.tile_pool(name="qsl", bufs=2) as qp,
            tc.tile_pool(name="expp", bufs=2) as epool,
            tc.tile_pool(name="outp", bufs=2) as opool,
            tc.tile_pool(name="rcp", bufs=4) as rpool,
            tc.tile_pool(name="psc", bufs=3, space="PSUM") as psc,
            tc.tile_pool(name="pctx", bufs=2, space="PSUM") as pctx,
            tc.tile_pool(name="prs", bufs=2, space="PSUM") as prs,
        ):
            def load_q_slice(sb):
                q_sl = qp.tile([PD, DK, SBLK], F32R, name="q_sl", tag="q_sl")
                nc.sync.dma_start(
                    out=q_sl,
                    in_=qT_dram[:, sb * SBLK : (sb + 1) * SBLK].rearrange(
                        "(dk p) s -> p dk s", p=PD
                    ),
                )
                return q_sl

            q_next = load_q_slice(0)
            v_sb = vp.tile([PD, SC, D], F32R)
            vr = v_dram.rearrange("(c p) d -> p c d", p=PD)
            for c4 in range(4):
                nc.sync.dma_start(
                    out=v_sb[:, c4 * 4 : (c4 + 1) * 4, :],
                    in_=vr[:, c4 * 4 : (c4 + 1) * 4, :],
                )
            for sb in range(NBLK):
                q_sl = q_next
                exp_sb = epool.tile([PD, SC, SBLK], F32R)
                for tcn in range(SC):
                    ps = psc.tile([PD, SBLK], F32)
                    for dk in range(DK):
                        nc.tensor.matmul(
                            out=ps,
                            lhsT=kT[:, dk, tcn * PD : (tcn + 1) * PD],
                            rhs=q_sl[:, dk, :],
                            start=(dk == 0),
                            stop=(dk == DK - 1),
                        )
                    nc.scalar.activation(
                        out=exp_sb[:, tcn, :],
                        in_=ps,
                        func=EXP,
                        scale=0.125,
                        bias=mask_sb[:, tcn : tcn + 1],
                    )
                if sb + 1 < NBLK:
                    q_next = load_q_slice(sb + 1)
                # rowsum over t: ones as the stationary operand (1-col LDW),
                # giving rowsumT [1, SBLK]; then per-128 transpose via PE
                # (identity [1,1]) to get per-partition [128,1] reciprocals.
                psr = prs.tile([1, SBLK], F32, bufs=1)
                for tcn in range(SC):
                    nc.tensor.matmul(
                        out=psr,
                        lhsT=ones_r[:, 0:1],
                        rhs=exp_sb[:, tcn, :],
                        start=(tcn == 0),
                        stop=(tcn == SC - 1),
                    )
                rs_sb = rpool.tile([1, SBLK], F32, name="rs_sb", tag="rs_sb")
                nc.vector.tensor_copy(out=rs_sb, in_=psr)
                recips = []
                for ss in range(SBLK // PD):
                    ptp = prs.tile([PD, 1], F32, name="ptp", tag="ptp", bufs=2)
                    nc.tensor.transpose(
                        out=ptp,
                        in_=rs_sb[0:1, ss * PD : (ss + 1) * PD],
                        identity=ident[0:1, 0:1],
                    )
                    recip_t = rpool.tile([PD, 1], F32, name="recip_t", tag="recip_t")
                    nc.vector.reciprocal(out=recip_t, in_=ptp)
                    recips.append(recip_t)
                for ss in range(SBLK // PD):
                    recip = recips[ss]
                    ostage = opool.tile([PD, D], F32)
                    for dt in range(D // NT):
                        pc = pctx.tile([PD, NT], F32)
                        for tcn in range(SC):
                            nc.tensor.matmul(
                                out=pc,
                                lhsT=exp_sb[:, tcn, ss * PD : (ss + 1) * PD],
                                rhs=v_sb[:, tcn, dt * NT : (dt + 1) * NT],
                                start=(tcn == 0),
                                stop=(tcn == SC - 1),
                            )
                        nc.vector.tensor_scalar_mul(
                            out=ostage[:, dt * NT : (dt + 1) * NT],
                            in0=pc,
                            scalar1=recip,
                        )
                    row = sb * SBLK + ss * PD
                    nc.sync.dma_start(out=out[row : row + PD, :], in_=ostage)

    nc.compile()
    return nc


def _get_compiled():
    global _compiled_nc
    if _compiled_nc is None:
        _compiled_nc = _build()
    return _compiled_nc


def _run(inputs, **kwargs):
    hs = np.asarray(inputs["hidden_states"], dtype=np.float32)
    mask = np.asarray(inputs["attention_mask"], dtype=np.float32)
    ws = {
        k: np.ascontiguousarray(np.asarray(inputs[k], dtype=np.float32))
        for k in ("Wq", "bq", "Wk", "bk", "Wv", "bv")
    }
    nc = _get_compiled()
    in_maps = [
        {
            "hidden_states": np.ascontiguousarray(hs[i]),
            "attention_mask": np.ascontiguousarray(mask[i]),
            **ws,
        }
        for i in range(NCORES)
    ]
    r = run_bass_kernel_spmd(nc, in_maps, list(range(NCORES)), **kwargs)
    out = np.stack([r.results[i]["context"] for i in range(NCORES)], axis=0)
    return out, r


def kernel(**inputs) -> np.ndarray:
    out, _ = _run(inputs)
    return out


if __name__ == "__main__":
    rng = np.random.default_rng(0)
    scale = 1.0 / np.sqrt(D)
    inputs = {
        "hidden_states": rng.standard_normal((B, S, D)).astype(np.float32),
        "attention_mask": np.zeros((B, 1, S), np.float32),
        "Wq": (rng.standard_normal((D, D)) * scale).astype(np.float32),
        "bq": np.zeros(D, np.float32),
        "Wk": (rng.standard_normal((D, D)) * scale).astype(np.float32),
        "bk": np.zeros(D, np.float32),
        "Wv": (rng.standard_normal((D, D)) * scale).astype(np.float32),
        "bv": np.zeros(D, np.float32),
    }
    got = kernel(**inputs)

    hs64 = inputs["hidden_states"].astype(np.float64)
    q = hs64 @ inputs["Wq"].astype(np.float64)
    k = hs64 @ inputs["Wk"].astype(np.float64)
    v = hs64 @ inputs["Wv"].astype(np.float64)
    sc = np.einsum("bsd,btd->bst", q, k) / 8.0
    sc -= sc.max(axis=-1, keepdims=True)
    p = np.exp(sc)
    p /= p.sum(axis=-1, keepdims=True)
    ref = np.einsum("bst,btd->bsd", p, v)
    err = np.abs(got.astype(np.float64) - ref)
    print(
        f"absmax={err.max():.3e} rel_vs_scale={err.max() / np.abs(ref).max():.3e} "
        f"rms_rel={np.sqrt((err**2).mean()) / np.sqrt((ref**2).mean()):.3e}"
    )



# revision 4
# speedup vs baseline: 1.0287x; 1.0287x over previous
"""Trainium2 Bass kernel for CustomBertSelfAttention (no head split).

reference:
    q = hs @ Wq + bq; k = hs @ Wk + bk; v = hs @ Wv + bv        # [B,S,D]
    scores = (q @ k^T) / sqrt(64) + mask                         # [B,S,S]
    probs  = softmax(scores, -1)
    out    = probs @ v                                           # [B,S,D]

B=8, S=2048, D=1024.  Sharding: data-parallel over batch, one batch
element per NeuronCore (8 cores), no collectives.

v2 plan — all matmul operands 16-bit (fp16 for hs/W/q/k, bf16 for
exp/v since exp values reach ~2e10 and overflow fp16), which keeps the
PE at 1 cycle/row like fp32r but:
  * everything fits SBUF resident (no DRAM spill round-trips)
  * FWL fast-weight-load applies (fp32r is excluded from FWL)
  * hs transpose runs as REGULAR matmuls (stationary=hs chunk,
    moving=identity) at 1 cycle/row instead of fp32 transpose-mode at
    2 cycles/row -- and real matmuls count as PE-busy for the HAM
    clock gate, so the transpose phase no longer runs cold.
  * rowsum is folded into the context matmul via ones-columns
    appended to v, landing [s-part, 1] in PSUM: no separate rowsum
    pass, no [1,N]->[N,1] transposes before the reciprocal.
  * biases/mask loaded as [c,128] rows (fast contiguous DMA) + one PE
    transpose each, instead of 7-14us DIRECT2D 4-byte scatters.
  * phase 2 emitted software-pipelined (S0 S1 C0 S2 C1 ...) so the PE
    never waits on the exp activation.

Measured numerics (numpy simulation of the exact rounding chain):
max rel-to-max-|out| error 4.2e-3 vs the 2e-2 gate.
"""

import sys

sys.path.insert(0, "/opt/trn_rl_repo")

from contextlib import ExitStack

import numpy as np

import concourse.bass as bass
import concourse.mybir as mybir
import concourse.tile as tile
from concourse import bacc
from concourse.bass_utils import run_bass_kernel_spmd
from concourse.masks import make_identity

B, S, D = 8, 2048, 1024
NCORES = 8
PD = 128            # partition dim
DK = D // PD        # 8 contraction chunks
SC = S // PD        # 16 sequence chunks
NT = 512            # matmul moving-dim tile (one PSUM bank of fp32)
SBLK = 512          # attention s-block
NBLK = S // SBLK    # 4
VW = D + 4          # v row width incl. ones cols for the fused rowsum
F32 = mybir.dt.float32
F16 = mybir.dt.float16
BF16 = mybir.dt.bfloat16
EXP = mybir.ActivationFunctionType.Exp

_compiled_nc = None


def _build():
    nc = bacc.Bacc(
        "TRN2",
        target_bir_lowering=False,
        debug=False,
        num_devices=NCORES,
        enable_asserts=False,
    )
    hs = nc.dram_tensor("hidden_states", [S, D], F32, kind="ExternalInput").ap()
    mask = nc.dram_tensor("attention_mask", [1, S], F32, kind="ExternalInput").ap()
    Wq = nc.dram_tensor("Wq", [D, D], F32, kind="ExternalInput").ap()
    Wk = nc.dram_tensor("Wk", [D, D], F32, kind="ExternalInput").ap()
    Wv = nc.dram_tensor("Wv", [D, D], F32, kind="ExternalInput").ap()
    bq = nc.dram_tensor("bq", [D], F32, kind="ExternalInput").ap()
    bk = nc.dram_tensor("bk", [D], F32, kind="ExternalInput").ap()
    bv = nc.dram_tensor("bv", [D], F32, kind="ExternalInput").ap()
    out = nc.dram_tensor("context", [S, D], F32, kind="ExternalOutput").ap()

    with tile.TileContext(nc) as tc, ExitStack() as ctx:
        persist = ctx.enter_context(tc.tile_pool(name="persist", bufs=1))
        dramp = ctx.enter_context(tc.tile_pool(name="dram", bufs=1, space="DRAM"))

        kT = persist.tile([PD, DK, S], F16)      # [e-part, m, t]
        qT = persist.tile([PD, DK, S], F16)      # [e-part, m, s]
        v_sb = persist.tile([PD, SC, VW], BF16)  # [t-part, c, d | ones]

        mask_sb = persist.tile([PD, SC], F32)    # bias per t-chunk for exp
        bq_sb = persist.tile([PD, DK], F32)
        bk_sb = persist.tile([PD, DK], F32)
        bv_row = persist.tile([PD, D], F32)

        ident = persist.tile([PD, PD], F32)
        make_identity(nc, ident)
        ident16 = persist.tile([PD, PD], F16)
        nc.vector.tensor_copy(out=ident16, in_=ident)

        with ExitStack() as p1:
            hstp = p1.enter_context(tc.tile_pool(name="hsT_pool", bufs=1))
            hsT = hstp.tile([PD, DK, S], F16)    # [d-part, dk, s]

            rowp = p1.enter_context(tc.tile_pool(name="rows", bufs=1))
            bk_row = rowp.tile([DK, PD], F32)
            bq_row = rowp.tile([DK, PD], F32)
            mask_row = rowp.tile([SC, PD], F32)

            w16p = p1.enter_context(tc.tile_pool(name="w16", bufs=2))
            wstp = p1.enter_context(tc.tile_pool(name="wst", bufs=2))

            # ---- phase 1a: hs -> hsT via regular fp16 matmuls
            with (
                tc.tile_pool(name="hsload", bufs=2) as hsp,
                tc.tile_pool(name="hs16", bufs=2) as h16p,
                tc.tile_pool(name="ptr", bufs=2, space="PSUM") as ptr,
                tc.tile_pool(name="pbias", bufs=1, space="PSUM") as pbias,
                tc.tile_pool(name="junkp", bufs=1, space="PSUM") as junkp,
                tc.tile_pool(name="junks", bufs=1) as junksp,
            ):
                # ~2.6us of junk fp16 matmuls while the first hs chunk
                # loads: warms the HAM clock gate so the transpose matmuls
                # and first projections run at 2.4 GHz.  DMA-out so DCE
                # keeps it.
                warm_ps = junkp.tile([PD, NT], F32, name="warm_ps", tag="warm_ps")
                for _ in range(24):
                    nc.tensor.matmul(
                        out=warm_ps[:, 0:PD],
                        lhsT=ident16,
                        rhs=ident16,
                        start=True,
                        stop=True,
                    )
                warm_sb = junksp.tile([PD, PD], F32, name="warm_sb", tag="warm_sb")
                nc.vector.tensor_copy(out=warm_sb, in_=warm_ps[:, 0:PD])
                warm_dram = dramp.tile([PD, PD], F32, name="warm_dram", tag="warm_dram")
                nc.sync.dma_start(out=warm_dram[:, :], in_=warm_sb)

                for sc in range(SC):
                    hchunk = hsp.tile([PD, D], F32)
                    nc.sync.dma_start(out=hchunk, in_=hs[sc * PD : (sc + 1) * PD, :])
                    if sc == 1:
                        # fast contiguous row loads for the biases
                        nc.sync.dma_start(
                            out=bk_row, in_=bk.rearrange("(c p) -> c p", c=DK)
                        )
                        nc.sync.dma_start(
                            out=bq_row, in_=bq.rearrange("(c p) -> c p", c=DK)
                        )
                    if sc == 8:
                        nc.sync.dma_start(
                            out=mask_row,
                            in_=mask[0, :].rearrange("(c p) -> c p", c=SC),
                        )
                    if sc == SC - 1:
                        # broadcast bv across partitions (slow DIRECT2D
                        # replication ~11us, but the sync queue is idle
                        # from here until the first output at ~200us)
                        bv_bcast = bass.AP(
                            tensor=bv.tensor, offset=bv.offset, ap=[[0, PD], *bv.ap]
                        )
                        nc.sync.dma_start(out=bv_row, in_=bv_bcast)
                    h16 = h16p.tile([PD, D], F16)
                    nc.scalar.copy(out=h16, in_=hchunk)
                    for half in range(2):
                        pst = ptr.tile([PD, 4, PD], F32)
                        for j in range(4):
                            dk = half * 4 + j
                            nc.tensor.matmul(
                                out=pst[:, j, :],
                                lhsT=h16[:, dk * PD : (dk + 1) * PD],
                                rhs=ident16,
                                start=True,
                                stop=True,
                            )
                        nc.vector.tensor_copy(
                            out=hsT[
                                :, half * 4 : (half + 1) * 4, sc * PD : (sc + 1) * PD
                            ],
                            in_=pst,
                        )
                    if sc == 3:
                        # bias rows -> [128, DK] via one PE transpose each
                        pb = pbias.tile([PD, NT], F32, name="pb_k", tag="pbias")
                        nc.tensor.transpose(
                            out=pb[:, 0:DK], in_=bk_row, identity=ident[0:DK, 0:DK]
                        )
                        nc.vector.tensor_copy(out=bk_sb, in_=pb[:, 0:DK])
                        pb2 = pbias.tile([PD, NT], F32, name="pb_q", tag="pbias")
                        nc.tensor.transpose(
                            out=pb2[:, 0:DK], in_=bq_row, identity=ident[0:DK, 0:DK]
                        )
                        nc.vector.tensor_copy(out=bq_sb, in_=pb2[:, 0:DK])
                    if sc == 9:
                        pbm = pbias.tile([PD, NT], F32, name="pb_m", tag="pbias")
                        nc.tensor.transpose(
                            out=pbm[:, 0:SC], in_=mask_row, identity=ident[0:SC, 0:SC]
                        )
                        nc.vector.tensor_copy(out=mask_sb, in_=pbm[:, 0:SC])

            # ---- W loads: column stripes (f32 stage -> f16), gpsimd queue
            def load_w16(W):
                w16 = w16p.tile([PD, DK, D], F16, name="w16_t", tag="w16_t")
                for m in range(DK):
                    wst = wstp.tile([PD, DK, PD], F32, name="wst_t", tag="wst_t")
                    nc.gpsimd.dma_start(
                        out=wst,
                        in_=W.rearrange("(dk p) n -> p dk n", p=PD)[
                            :, :, m * PD : (m + 1) * PD
                        ],
                    )
                    nc.vector.tensor_copy(
                        out=w16[:, :, m * PD : (m + 1) * PD], in_=wst
                    )
                return w16

            wk16 = load_w16(Wk)
            wq16 = load_w16(Wq)

            # ---- phase 1b: projections
            def project_qk(w16, bias_sb, dst):
                with tc.tile_pool(name="pp", bufs=3, space="PSUM") as pp:
                    for m in range(DK):
                        for tb in range(S // NT):
                            ps = pp.tile([PD, NT], F32)
                            for dk in range(DK):
                                nc.tensor.matmul(
                                    out=ps,
                                    lhsT=w16[:, dk, m * PD : (m + 1) * PD],
                                    rhs=hsT[:, dk, tb * NT : (tb + 1) * NT],
                                    start=(dk == 0),
                                    stop=(dk == DK - 1),
                                )
                            nc.vector.tensor_scalar_add(
                                out=dst[:, m, tb * NT : (tb + 1) * NT],
                                in0=ps,
                                scalar1=bias_sb[:, m : m + 1],
                            )

            project_qk(wk16, bk_sb, kT)
            project_qk(wq16, bq_sb, qT)

            wv16 = load_w16(Wv)
            nc.vector.memset(v_sb[:, :, D:VW], 1.0)
            with tc.tile_pool(name="pv", bufs=3, space="PSUM") as pv:
                for c in range(SC):
                    for dt in range(D // NT):
                        ps = pv.tile([PD, NT], F32)
                        for dk in range(DK):
                            nc.tensor.matmul(
                                out=ps,
                                lhsT=hsT[:, dk, c * PD : (c + 1) * PD],
                                rhs=wv16[:, dk, dt * NT : (dt + 1) * NT],
                                start=(dk == 0),
                                stop=(dk == DK - 1),
                            )
                        nc.vector.tensor_add(
                            out=v_sb[:, c, dt * NT : (dt + 1) * NT],
                            in0=ps,
                            in1=bv_row[:, dt * NT : (dt + 1) * NT],
                        )

        # ---- phase 2: attention, software-pipelined S0 S1 C0 S2 C1 ...
        with (
            tc.tile_pool(name="expp", bufs=3) as epool,
            tc.tile_pool(name="outp", bufs=2) as opool,
            tc.tile_pool(name="rcp", bufs=4) as rpool,
            tc.tile_pool(name="psc", bufs=2, space="PSUM") as psc,
            tc.tile_pool(name="pca", bufs=2, space="PSUM") as pca,
            tc.tile_pool(name="pcb", bufs=2, space="PSUM") as pcb,
            tc.tile_pool(name="pcr", bufs=2, space="PSUM") as pcr,
        ):
            def scores_block(sb):
                exp_sb = epool.tile(
                    [PD, SC, SBLK], BF16, name="exp_sb", tag="exp_sb"
                )
                for tcn in range(SC):
                    ps = psc.tile([PD, SBLK], F32)
                    for dk in range(DK):
                        nc.tensor.matmul(
                            out=ps,
                            lhsT=kT[:, dk, tcn * PD : (tcn + 1) * PD],
                            rhs=qT[:, dk, sb * SBLK : (sb + 1) * SBLK],
                            start=(dk == 0),
                            stop=(dk == DK - 1),
                        )
                    nc.scalar.activation(
                        out=exp_sb[:, tcn, :],
                        in_=ps,
                        func=EXP,
                        scale=0.125,
                        bias=mask_sb[:, tcn : tcn + 1],
                    )
                return exp_sb

            def context_block(sb, exp_sb):
                for ss in range(SBLK // PD):
                    pa = pca.tile([PD, NT], F32)
                    pb = pcb.tile([PD, NT], F32)
                    pr = pcr.tile([PD, NT], F32)
                    for tcn in range(SC):
                        st, sp = (tcn == 0), (tcn == SC - 1)
                        e_sl = exp_sb[:, tcn, ss * PD : (ss + 1) * PD]
                        nc.tensor.matmul(
                            out=pa, lhsT=e_sl, rhs=v_sb[:, tcn, 0:NT],
                            start=st, stop=sp,
                        )
                        nc.tensor.matmul(
                            out=pb, lhsT=e_sl, rhs=v_sb[:, tcn, NT : 2 * NT],
                            start=st, stop=sp,
                        )
                        nc.tensor.matmul(
                            out=pr[:, 0:4], lhsT=e_sl, rhs=v_sb[:, tcn, D:VW],
                            start=st, stop=sp,
                        )
                    recip = rpool.tile([PD, 1], F32, name="recip_t", tag="recip_t")
                    nc.vector.reciprocal(out=recip, in_=pr[:, 0:1])
                    ostage = opool.tile([PD, D], F32)
                    nc.vector.tensor_scalar_mul(
                        out=ostage[:, 0:NT], in0=pa, scalar1=recip
                    )
                    nc.vector.tensor_scalar_mul(
                        out=ostage[:, NT : 2 * NT], in0=pb, scalar1=recip
                    )
                    row = sb * SBLK + ss * PD
                    nc.sync.dma_start(out=out[row : row + PD, :], in_=ostage)

            pending = []
            for sb in range(NBLK):
                e = scores_block(sb)
                pending.append((sb, e))
                if sb >= 1:
                    context_block(*pending.pop(0))
            while pending:
                context_block(*pending.pop(0))

    nc.compile()
    return nc


def _get_compiled():
    global _compiled_nc
    if _compiled_nc is None:
        _compiled_nc = _build()
    return _compiled_nc


def _run(inputs, **kwargs):
    hs = np.asarray(inputs["hidden_states"], dtype=np.float32)
    mask = np.asarray(inputs["attention_mask"], dtype=np.float32)
    ws = {
        k: np.ascontiguousarray(np.asarray(inputs[k], dtype=np.float32))
        for k in ("Wq", "bq", "Wk", "bk", "Wv", "bv")
    }
    nc = _get_compiled()
    in_maps = [
        {
            "hidden_states": np.ascontiguousarray(hs[i]),
            "attention_mask": np.ascontiguousarray(mask[i]),
            **ws,
        }
        for i in range(NCORES)
    ]
    r = run_bass_kernel_spmd(nc, in_maps, list(range(NCORES)), **kwargs)
    out = np.stack([r.results[i]["context"] for i in range(NCORES)], axis=0)
    return out, r


def kernel(**inputs) -> np.ndarray:
    out, _ = _run(inputs)
    return out


if __name__ == "__main__":
    rng = np.random.default_rng(0)
    scale = 1.0 / np.sqrt(D)
    inputs = {
        "hidden_states": rng.standard_normal((B, S, D)).astype(np.float32),
        "attention_mask": np.zeros((B, 1, S), np.float32),
        "Wq": (rng.standard_normal((D, D)) * scale).astype(np.float32),
        "bq": np.zeros(D, np.float32),
        "Wk": (rng.standard_normal((D, D)) * scale).astype(np.float32),
        "bk": np.zeros(D, np.float32),
        "Wv": (rng.standard_normal((D, D)) * scale).astype(np.float32),
        "bv": np.zeros(D, np.float32),
    }
    got = kernel(**inputs)

    hs64 = inputs["hidden_states"].astype(np.float64)
    q = hs64 @ inputs["Wq"].astype(np.float64)
    k = hs64 @ inputs["Wk"].astype(np.float64)
    v = hs64 @ inputs["Wv"].astype(np.float64)
    sc = np.einsum("bsd,btd->bst", q, k) / 8.0
    sc -= sc.max(axis=-1, keepdims=True)
    p = np.exp(sc)
    p /= p.sum(axis=-1, keepdims=True)
    ref = np.einsum("bst,btd->bsd", p, v)
    err = np.abs(got.astype(np.float64) - ref)
    print(
        f"absmax={err.max():.3e} rel_vs_scale={err.max() / np.abs(ref).max():.3e} "
        f"rms_rel={np.sqrt((err**2).mean()) / np.sqrt((ref**2).mean()):.3e}"
    )


# revision 12
# speedup vs baseline: 1.1235x; 1.0922x over previous
"""Trainium2 Bass kernel for CustomBertSelfAttention (no head split).

reference:
    q = hs @ Wq + bq; k = hs @ Wk + bk; v = hs @ Wv + bv        # [B,S,D]
    scores = (q @ k^T) / sqrt(64) + mask                         # [B,S,S]
    probs  = softmax(scores, -1)
    out    = probs @ v                                           # [B,S,D]

B=8, S=2048, D=1024.  Sharding: data-parallel over batch, one batch
element per NeuronCore (8 cores), no collectives.

v2 plan — all matmul operands 16-bit (fp16 for hs/W/q/k, bf16 for
exp/v since exp values reach ~2e10 and overflow fp16), which keeps the
PE at 1 cycle/row like fp32r but:
  * everything fits SBUF resident (no DRAM spill round-trips)
  * FWL fast-weight-load applies (fp32r is excluded from FWL)
  * hs transpose runs as REGULAR matmuls (stationary=hs chunk,
    moving=identity) at 1 cycle/row instead of fp32 transpose-mode at
    2 cycles/row -- and real matmuls count as PE-busy for the HAM
    clock gate, so the transpose phase no longer runs cold.
  * rowsum is folded into the context matmul via ones-columns
    appended to v, landing [s-part, 1] in PSUM: no separate rowsum
    pass, no [1,N]->[N,1] transposes before the reciprocal.
  * biases/mask loaded as [c,128] rows (fast contiguous DMA) + one PE
    transpose each, instead of 7-14us DIRECT2D 4-byte scatters.
  * phase 2 emitted software-pipelined (S0 S1 C0 S2 C1 ...) so the PE
    never waits on the exp activation.

Measured numerics (numpy simulation of the exact rounding chain):
max rel-to-max-|out| error 4.2e-3 vs the 2e-2 gate.
"""

import sys

sys.path.insert(0, "/opt/trn_rl_repo")

from contextlib import ExitStack

import numpy as np

import concourse.bass as bass
import concourse.mybir as mybir
import concourse.tile as tile
from concourse import bacc
from concourse.bass_utils import run_bass_kernel_spmd
from concourse.masks import make_identity

B, S, D = 8, 2048, 1024
NCORES = 8
PD = 128            # partition dim
DK = D // PD        # 8 contraction chunks
SC = S // PD        # 16 sequence chunks
NT = 512            # matmul moving-dim tile (one PSUM bank of fp32)
SBLK = 512          # attention s-block
NBLK = S // SBLK    # 4
VW = D + 4          # v row width incl. ones cols for the fused rowsum
F32 = mybir.dt.float32
F16 = mybir.dt.float16
BF16 = mybir.dt.bfloat16
EXP = mybir.ActivationFunctionType.Exp

_compiled_nc = None


def _build():
    nc = bacc.Bacc(
        "TRN2",
        target_bir_lowering=False,
        debug=False,
        num_devices=NCORES,
        enable_asserts=False,
    )
    hs = nc.dram_tensor("hidden_states", [S, D], F32, kind="ExternalInput").ap()
    mask = nc.dram_tensor("attention_mask", [1, S], F32, kind="ExternalInput").ap()
    Wq = nc.dram_tensor("Wq", [D, D], F32, kind="ExternalInput").ap()
    Wk = nc.dram_tensor("Wk", [D, D], F32, kind="ExternalInput").ap()
    Wv = nc.dram_tensor("Wv", [D, D], F32, kind="ExternalInput").ap()
    bq = nc.dram_tensor("bq", [D], F32, kind="ExternalInput").ap()
    bk = nc.dram_tensor("bk", [D], F32, kind="ExternalInput").ap()
    bv = nc.dram_tensor("bv", [D], F32, kind="ExternalInput").ap()
    out = nc.dram_tensor("context", [S, D], F32, kind="ExternalOutput").ap()

    with tile.TileContext(nc) as tc, ExitStack() as ctx:
        persist = ctx.enter_context(tc.tile_pool(name="persist", bufs=1))
        dramp = ctx.enter_context(tc.tile_pool(name="dram", bufs=1, space="DRAM"))

        kT = persist.tile([PD, DK, S], F16)      # [e-part, m, t]
        qT = persist.tile([PD, DK, S], F16)      # [e-part, m, s]
        v_sb = persist.tile([PD, SC, VW], BF16)  # [t-part, c, d | ones]

        mask_sb = persist.tile([PD, SC], F32)    # bias per t-chunk for exp
        bq_sb = persist.tile([PD, DK], F32)
        bk_sb = persist.tile([PD, DK], F32)
        bv_row = persist.tile([PD, D], F32)

        ident = persist.tile([PD, PD], F32)
        make_identity(nc, ident)
        ident16 = persist.tile([PD, PD], F16)
        nc.vector.tensor_copy(out=ident16, in_=ident)

        with ExitStack() as p1:
            hstp = p1.enter_context(tc.tile_pool(name="hsT_pool", bufs=1))
            hsT = hstp.tile([PD, DK, S], F16)    # [d-part, dk, s]

            rowp = p1.enter_context(tc.tile_pool(name="rows", bufs=1))
            bk_row = rowp.tile([DK, PD], F32)
            bq_row = rowp.tile([DK, PD], F32)
            mask_row = rowp.tile([SC, PD], F32)

            w16p = p1.enter_context(tc.tile_pool(name="w16", bufs=2))
            wstp = p1.enter_context(tc.tile_pool(name="wst", bufs=2))
            hsp = p1.enter_context(tc.tile_pool(name="hsload", bufs=3))
            h16p = p1.enter_context(tc.tile_pool(name="hs16", bufs=2))
            ptr = p1.enter_context(tc.tile_pool(name="ptr", bufs=4, space="PSUM"))
            pbias = p1.enter_context(
                tc.tile_pool(name="pbias", bufs=1, space="PSUM")
            )
            junkp = p1.enter_context(
                tc.tile_pool(name="junkp", bufs=1, space="PSUM")
            )
            junksp = p1.enter_context(tc.tile_pool(name="junks", bufs=1))
            pp = p1.enter_context(tc.tile_pool(name="pp", bufs=2, space="PSUM"))

            # DMA emission is software-pipelined: a buffer-rotating pool
            # tile may only get its next DMA writer emitted AFTER the
            # previous epoch's readers are emitted (Tile dep tracking is
            # forward-only), so hs chunk sc+3 is issued when chunk sc is
            # consumed, and W quarter DMAs are issued one per completed
            # W-quarter cast.
            hchunks = {}

            def issue_hs_dma(sc):
                if sc >= SC:
                    return
                hchunk = hsp.tile([PD, D], F32, name="hchunk", tag="hchunk")
                nc.sync.dma_start(out=hchunk, in_=hs[sc * PD : (sc + 1) * PD, :])
                hchunks[sc] = hchunk
                if sc == 1:
                    nc.sync.dma_start(
                        out=bk_row, in_=bk.rearrange("(c p) -> c p", c=DK)
                    )
                    nc.sync.dma_start(
                        out=bq_row, in_=bq.rearrange("(c p) -> c p", c=DK)
                    )
                if sc == 8:
                    nc.sync.dma_start(
                        out=mask_row,
                        in_=mask[0, :].rearrange("(c p) -> c p", c=SC),
                    )
                if sc == SC - 1:
                    # broadcast bv across partitions (slow DIRECT2D
                    # replication ~11us, but the sync queue is idle from
                    # here until the first output at ~200us)
                    bv_bcast = bass.AP(
                        tensor=bv.tensor, offset=bv.offset, ap=[[0, PD], *bv.ap]
                    )
                    nc.sync.dma_start(out=bv_row, in_=bv_bcast)

            # W quarter queue: (target-f16-tile-getter, quarter) in load order
            w_dma_plan = [(Wk, qn) for qn in range(4)]
            w_dma_plan += [(Wq, qn) for qn in range(4)]
            w_dma_plan += [(Wv, qn) for qn in range(4)]
            w_stages = {}
            w_dma_pos = 0

            def issue_w_dma():
                nonlocal w_dma_pos
                if w_dma_pos >= len(w_dma_plan):
                    return
                W, qn = w_dma_plan[w_dma_pos]
                w_dma_pos += 1
                wst = wstp.tile([PD, 2, D], F32, name="wst_t", tag="wst_t")
                nc.gpsimd.dma_start(
                    out=wst,
                    in_=W.rearrange("(dk p) n -> p dk n", p=PD)[
                        :, 2 * qn : 2 * qn + 2, :
                    ],
                )
                w_stages[(id(W), qn)] = wst

            def cast_w_quarter(w16, W, qn):
                nc.vector.tensor_copy(
                    out=w16[:, 2 * qn : 2 * qn + 2, :],
                    in_=w_stages.pop((id(W), qn)),
                )
                issue_w_dma()

            issue_hs_dma(0)
            issue_hs_dma(1)
            issue_hs_dma(2)
            issue_w_dma()
            issue_w_dma()
            wk16 = w16p.tile([PD, DK, D], F16, name="w16_t", tag="w16_t")
            wq16 = w16p.tile([PD, DK, D], F16, name="w16_t", tag="w16_t")

            # ---- PE warmup: ~70 junk matmuls on a memset tile (no
            # gpsimd dependency, so the PE is busy from ~1.3us and the HAM
            # clock gate opens before the real work arrives).
            jnk16 = junksp.tile([PD, 256], F16, name="jnk16", tag="jnk16")
            nc.vector.memset(jnk16, 0.25)
            warm_ps = junkp.tile([PD, NT], F32, name="warm_ps", tag="warm_ps")
            for _ in range(70):
                nc.tensor.matmul(
                    out=warm_ps[:, 0:256],
                    lhsT=jnk16[:, 0:PD],
                    rhs=jnk16,
                    start=True,
                    stop=True,
                )
            warm_sb = junksp.tile([PD, PD], F32, name="warm_sb", tag="warm_sb")
            nc.vector.tensor_copy(out=warm_sb, in_=warm_ps[:, 0:PD])
            warm_dram = dramp.tile([PD, PD], F32, name="warm_dram", tag="warm_dram")
            nc.sync.dma_start(out=warm_dram[:, :], in_=warm_sb)

            # ---- interleaved transpose + projection emission ----
            def transpose_chunk(sc):
                h16 = h16p.tile([PD, D], F16, name="h16", tag="h16")
                nc.scalar.copy(out=h16, in_=hchunks.pop(sc))
                issue_hs_dma(sc + 3)
                for half in range(2):
                    pst = ptr.tile([PD, 4, PD], F32)
                    for j in range(4):
                        dk = half * 4 + j
                        nc.tensor.matmul(
                            out=pst[:, j, :],
                            lhsT=h16[:, dk * PD : (dk + 1) * PD],
                            rhs=ident16,
                            start=True,
                            stop=True,
                        )
                    nc.vector.tensor_copy(
                        out=hsT[
                            :, half * 4 : (half + 1) * 4, sc * PD : (sc + 1) * PD
                        ],
                        in_=pst,
                    )

            def proj_block(w16, bias_sb, dst, tb):
                for m in range(DK):
                    ps = pp.tile([PD, NT], F32)
                    for dk in range(DK):
                        nc.tensor.matmul(
                            out=ps,
                            lhsT=w16[:, dk, m * PD : (m + 1) * PD],
                            rhs=hsT[:, dk, tb * NT : (tb + 1) * NT],
                            start=(dk == 0),
                            stop=(dk == DK - 1),
                        )
                    nc.vector.tensor_scalar_add(
                        out=dst[:, m, tb * NT : (tb + 1) * NT],
                        in0=ps,
                        scalar1=bias_sb[:, m : m + 1],
                    )

            for sc in range(4):
                transpose_chunk(sc)
                cast_w_quarter(wk16, Wk, sc)
            # bias rows -> [128, DK] via one PE transpose each
            pb = pbias.tile([PD, NT], F32, name="pb_k", tag="pbias")
            nc.tensor.transpose(
                out=pb[:, 0:DK], in_=bk_row, identity=ident[0:DK, 0:DK]
            )
            nc.vector.tensor_copy(out=bk_sb, in_=pb[:, 0:DK])
            pb2 = pbias.tile([PD, NT], F32, name="pb_q", tag="pbias")
            nc.tensor.transpose(
                out=pb2[:, 0:DK], in_=bq_row, identity=ident[0:DK, 0:DK]
            )
            nc.vector.tensor_copy(out=bq_sb, in_=pb2[:, 0:DK])
            for sc in range(4, 8):
                transpose_chunk(sc)

            proj_block(wk16, bk_sb, kT, 0)
            cast_w_quarter(wq16, Wq, 0)
            cast_w_quarter(wq16, Wq, 1)
            for sc in range(8, 12):
                transpose_chunk(sc)
            pbm = pbias.tile([PD, NT], F32, name="pb_m", tag="pbias")
            nc.tensor.transpose(
                out=pbm[:, 0:SC], in_=mask_row, identity=ident[0:SC, 0:SC]
            )
            nc.vector.tensor_copy(out=mask_sb, in_=pbm[:, 0:SC])

            proj_block(wk16, bk_sb, kT, 1)
            cast_w_quarter(wq16, Wq, 2)
            cast_w_quarter(wq16, Wq, 3)
            for sc in range(12, 16):
                transpose_chunk(sc)
            proj_block(wk16, bk_sb, kT, 2)
            proj_block(wk16, bk_sb, kT, 3)

            for tb in range(4):
                proj_block(wq16, bq_sb, qT, tb)
                if tb == 0:
                    # wv reuses wk16's buffer once the k projection is done
                    wv16 = w16p.tile([PD, DK, D], F16, name="w16_t", tag="w16_t")
                    for qn in range(4):
                        cast_w_quarter(wv16, Wv, qn)
                    nc.vector.memset(v_sb[:, :, D:VW], 1.0)

            for c in range(SC):
                for dt in range(D // NT):
                    ps = pp.tile([PD, NT], F32)
                    for dk in range(DK):
                        nc.tensor.matmul(
                            out=ps,
                            lhsT=hsT[:, dk, c * PD : (c + 1) * PD],
                            rhs=wv16[:, dk, dt * NT : (dt + 1) * NT],
                            start=(dk == 0),
                            stop=(dk == DK - 1),
                        )
                    nc.vector.tensor_add(
                        out=v_sb[:, c, dt * NT : (dt + 1) * NT],
                        in0=ps,
                        in1=bv_row[:, dt * NT : (dt + 1) * NT],
                    )

        # ---- phase 2: attention, software-pipelined S0 S1 C0 S2 C1 ...
        with (
            tc.tile_pool(name="expp", bufs=3) as epool,
            tc.tile_pool(name="outp", bufs=2) as opool,
            tc.tile_pool(name="rcp", bufs=4) as rpool,
            tc.tile_pool(name="psc", bufs=2, space="PSUM") as psc,
            tc.tile_pool(name="pca", bufs=2, space="PSUM") as pca,
            tc.tile_pool(name="pcb", bufs=2, space="PSUM") as pcb,
            tc.tile_pool(name="pcr", bufs=2, space="PSUM") as pcr,
        ):
            def scores_block(sb):
                exp_sb = epool.tile(
                    [PD, SC, SBLK], BF16, name="exp_sb", tag="exp_sb"
                )
                for tcn in range(SC):
                    ps = psc.tile([PD, SBLK], F32)
                    for dk in range(DK):
                        nc.tensor.matmul(
                            out=ps,
                            lhsT=kT[:, dk, tcn * PD : (tcn + 1) * PD],
                            rhs=qT[:, dk, sb * SBLK : (sb + 1) * SBLK],
                            start=(dk == 0),
                            stop=(dk == DK - 1),
                        )
                    nc.scalar.activation(
                        out=exp_sb[:, tcn, :],
                        in_=ps,
                        func=EXP,
                        scale=0.125,
                        bias=mask_sb[:, tcn : tcn + 1],
                    )
                return exp_sb

            def context_block(sb, exp_sb):
                for ss in range(SBLK // PD):
                    pa = pca.tile([PD, NT], F32)
                    pb = pcb.tile([PD, NT], F32)
                    pr = pcr.tile([PD, NT], F32)
                    for tcn in range(SC):
                        st, sp = (tcn == 0), (tcn == SC - 1)
                        e_sl = exp_sb[:, tcn, ss * PD : (ss + 1) * PD]
                        nc.tensor.matmul(
                            out=pa, lhsT=e_sl, rhs=v_sb[:, tcn, 0:NT],
                            start=st, stop=sp,
                        )
                        nc.tensor.matmul(
                            out=pb, lhsT=e_sl, rhs=v_sb[:, tcn, NT : 2 * NT],
                            start=st, stop=sp,
                        )
                        nc.tensor.matmul(
                            out=pr[:, 0:4], lhsT=e_sl, rhs=v_sb[:, tcn, D:VW],
                            start=st, stop=sp,
                        )
                    recip = rpool.tile([PD, 1], F32, name="recip_t", tag="recip_t")
                    nc.vector.reciprocal(out=recip, in_=pr[:, 0:1])
                    ostage = opool.tile([PD, D], F32)
                    nc.vector.tensor_scalar_mul(
                        out=ostage[:, 0:NT], in0=pa, scalar1=recip
                    )
                    nc.vector.tensor_scalar_mul(
                        out=ostage[:, NT : 2 * NT], in0=pb, scalar1=recip
                    )
                    row = sb * SBLK + ss * PD
                    nc.sync.dma_start(out=out[row : row + PD, :], in_=ostage)

            pending = []
            for sb in range(NBLK):
                e = scores_block(sb)
                pending.append((sb, e))
                if sb >= 1:
                    context_block(*pending.pop(0))
            while pending:
                context_block(*pending.pop(0))

    nc.compile()
    return nc


def _get_compiled():
    global _compiled_nc
    if _compiled_nc is None:
        _compiled_nc = _build()
    return _compiled_nc


def _run(inputs, **kwargs):
    hs = np.asarray(inputs["hidden_states"], dtype=np.float32)
    mask = np.asarray(inputs["attention_mask"], dtype=np.float32)
    ws = {
        k: np.ascontiguousarray(np.asarray(inputs[k], dtype=np.float32))
        for k in ("Wq", "bq", "Wk", "bk", "Wv", "bv")
    }
    nc = _get_compiled()
    in_maps = [
        {
            "hidden_states": np.ascontiguousarray(hs[i]),
            "attention_mask": np.ascontiguousarray(mask[i]),
            **ws,
        }
        for i in range(NCORES)
    ]
    r = run_bass_kernel_spmd(nc, in_maps, list(range(NCORES)), **kwargs)
    out = np.stack([r.results[i]["context"] for i in range(NCORES)], axis=0)
    return out, r


def kernel(**inputs) -> np.ndarray:
    out, _ = _run(inputs)
    return out


if __name__ == "__main__":
    rng = np.random.default_rng(0)
    scale = 1.0 / np.sqrt(D)
    inputs = {
        "hidden_states": rng.standard_normal((B, S, D)).astype(np.float32),
        "attention_mask": np.zeros((B, 1, S), np.float32),
        "Wq": (rng.standard_normal((D, D)) * scale).astype(np.float32),
        "bq": np.zeros(D, np.float32),
        "Wk": (rng.standard_normal((D, D)) * scale).astype(np.float32),
        "bk": np.zeros(D, np.float32),
        "Wv": (rng.standard_normal((D, D)) * scale).astype(np.float32),
        "bv": np.zeros(D, np.float32),
    }
    got = kernel(**inputs)

    hs64 = inputs["hidden_states"].astype(np.float64)
    q = hs64 @ inputs["Wq"].astype(np.float64)
    k = hs64 @ inputs["Wk"].astype(np.float64)
    v = hs64 @ inputs["Wv"].astype(np.float64)
    sc = np.einsum("bsd,btd->bst", q, k) / 8.0
    sc -= sc.max(axis=-1, keepdims=True)
    p = np.exp(sc)
    p /= p.sum(axis=-1, keepdims=True)
    ref = np.einsum("bst,btd->bsd", p, v)
    err = np.abs(got.astype(np.float64) - ref)
    print(
        f"absmax={err.max():.3e} rel_vs_scale={err.max() / np.abs(ref).max():.3e} "
        f"rms_rel={np.sqrt((err**2).mean()) / np.sqrt((ref**2).mean()):.3e}"
    )
